# revision 15
# baseline (speedup 1.0000x reference)
"""Trainium2 Bass kernel for nn_Attention_54778012893268 (v2: pipelined heads).

Fused QKV projection + RoPE + non-causal SDPA + output projection.
B=4, T=2048, C=2048, H=16, D=128, fp32 in/out.

Sharding: 8 cores = (batch b, head-group hg) pairs; b = core//2, hg = core%2.
Each core handles one batch's tokens and 8 of the 16 heads end-to-end,
producing a partial [T, C] output; the host sums the two head-group
partials per batch.

v2 design (vs v1):
- All matmul operands fp16 (same 1 col/cycle PE rate as bf16, 8x finer
  quantization). LDWEIGHTS for 2-byte stationaries is ~116ns and hides
  under the 213ns moving stream, so the per-matmul cadence is ~220ns
  (v1 fp32r stationaries were LDW-gated at 272ns).
- Everything SBUF-resident: x [C,T] fp16 (64KB/part), per-head q/k after
  RoPE, v for all heads, ynorm. No DRAM round-trips.
- Software-pipelined heads: attention of head h is interleaved (in PE
  program order) with the q/k projection of head h+1, so the Scalar
  engine's exp (the attention-phase bottleneck, ~13us/unit) hides under
  ~14.6us/unit of PE work.
- Softmax denominator off the PE: e-tiles are pair-summed on DVE (fp16,
  2x mode) and accumulated on GpSimd, leaving ONE [128,128]x[128,512]
  matmul per (head, chunk) instead of 16 (saves ~105us of PE time). The
  denom matmul + reciprocal + normalize for unit N are deferred into
  unit N+1's pipeline so the GpSimd chain never stalls the PE (-58us).
- Scores for two k-tiles land in one [128,1024] 2-bank PSUM tile (each
  matmul writes its own bank) so exp processes 1024 elements per
  ACTIVATE, amortizing the ~330ns PSUM-access init.
- Out-projection PSUMs share the qk-proj pool's bank budget; its first
  96 matmuls interleave into the last head's units to cover the
  no-more-projection tail.

Measured (NTFF profile, core 0): 712-714us, PE busy ~96% of span,
MFU-est 92.6%, rel err 9.6e-4 (tolerance 2e-2). Staged baseline
measured identically: 1043us. PE floor for this decomposition
(3104 required 512-col matmuls at 213ns) is ~660us; the remaining gap
is ~20us power throttling + ~25us of per-matmul fixed overhead + ramp.
"""

import math
import sys

import numpy as np

sys.path.insert(0, "/opt/trn_rl_repo")

P = 128
T = 2048
C = 2048
HPC = 8          # heads per core
D = 128
CH = 512         # T-chunk (PSUM bank width at fp32)
NCH = T // CH    # 4
KT = C // P      # 16 contraction tiles
TT = T // P      # 16 token tiles
SCALE = 1.0 / math.sqrt(D)
ROPE_BASE = 10000.0

_CACHED_NC = None


def build_nc():
    import concourse.bass as bass
    import concourse.tile as tile
    from concourse import bacc, mybir
    from contextlib import ExitStack

    F32 = mybir.dt.float32
    F16 = mybir.dt.float16
    ts = bass.ts

    nc = bacc.Bacc("TRN2", target_bir_lowering=False, debug=False, num_devices=8)

    xt = nc.dram_tensor("xt", [C, T], F16, kind="ExternalInput").ap()
    wq = nc.dram_tensor("wq", [C, HPC * D], F16, kind="ExternalInput").ap()
    wk = nc.dram_tensor("wk", [C, HPC * D], F16, kind="ExternalInput").ap()
    wv = nc.dram_tensor("wv", [C, HPC * D], F16, kind="ExternalInput").ap()
    wp = nc.dram_tensor("wp", [HPC * D, C], F16, kind="ExternalInput").ap()
    cosm = nc.dram_tensor("cosm", [P, T], F16, kind="ExternalInput").ap()
    sinm = nc.dram_tensor("sinm", [P, T], F16, kind="ExternalInput").ap()
    out = nc.dram_tensor("out", [T, C], F32, kind="ExternalOutput").ap()

    # pair-swap shuffle mask (within each 32-partition quadrant)
    SWAP_MASK = [i ^ 1 for i in range(32)]
    Exp = mybir.ActivationFunctionType.Exp

    with tile.TileContext(nc) as tc:
        with ExitStack() as outer:
            # ---- persistent SBUF pools (creation order = stack order) ----
            cpool = outer.enter_context(tc.tile_pool(name="const", bufs=1))
            mpool = outer.enter_context(tc.tile_pool(name="masks", bufs=1))
            vsp = outer.enter_context(tc.tile_pool(name="vsb", bufs=1))
            qkp = outer.enter_context(tc.tile_pool(name="qk", bufs=2))
            wqkp = outer.enter_context(tc.tile_pool(name="wqk", bufs=2))
            rpp = outer.enter_context(tc.tile_pool(name="rope", bufs=2))

            # PSUM: psQK first (lives whole kernel; outproj reuses its tag)
            psQK = outer.enter_context(
                tc.tile_pool(name="psQK", bufs=2, space="PSUM"))

            ones = cpool.tile([P, P], F16, tag="ones")
            nc.vector.memset(ones[:], 1.0)

            cos_sb = mpool.tile([P, T], F16, tag="cos")
            sin_sb = mpool.tile([P, T], F16, tag="sin")

            # v for all heads: v_sb[p, h, kt*128 + d] = v[key=kt*128+p, h, d]
            v_sb = vsp.tile([P, HPC, T], F16, tag="v")

            # x resident: xs[kt][p, t] = x[kt*128+p, t]
            es_x = ExitStack()
            xp = es_x.enter_context(tc.tile_pool(name="xs", bufs=1))
            xs = [xp.tile([P, T], F16, tag=f"xs{kt}", name=f"xs{kt}")
                  for kt in range(KT)]

            # phase-1-only pools (wv weights + vproj/warm psums)
            es1 = ExitStack()
            wvp = es1.enter_context(tc.tile_pool(name="wv", bufs=1))
            psV = es1.enter_context(tc.tile_pool(name="psV", bufs=3,
                                                 space="PSUM"))

            # ---- DMA priming ----
            wq_h = {}
            wk_h = {}

            def load_wqk(h):
                for wname, w_dram, store in (("wq", wq, wq_h), ("wk", wk, wk_h)):
                    wt = wqkp.tile([P, KT, P], F16, tag=wname,
                                   name=f"{wname}{h}")
                    nc.sync.dma_start(
                        wt[:],
                        w_dram[:, ts(h, P)].rearrange("(k p) d -> p k d", p=P),
                    )
                    store[h] = wt

            load_wqk(0)
            for kt in range(KT):
                nc.sync.dma_start(xs[kt][:, ts(0, CH)], xt[ts(kt, P), ts(0, CH)])
            wvt = []
            for kt in range(KT):
                wt = wvp.tile([P, HPC * D], F16, tag=f"wv{kt}", name=f"wv{kt}")
                nc.sync.dma_start(wt[:], wv[ts(kt, P), :])
                wvt.append(wt)
            # masks after wv: RoPE (DVE) tolerates the extra ~3us; the PE's
            # first vproj psum can't start until all of wv has landed
            nc.sync.dma_start(cos_sb[:], cosm)
            nc.sync.dma_start(sin_sb[:], sinm)
            for ci in range(1, NCH):
                for kt in range(KT):
                    nc.sync.dma_start(xs[kt][:, ts(ci, CH)],
                                      xt[ts(kt, P), ts(ci, CH)])
                if ci == 1:
                    load_wqk(1)

            # ---- warm the PE HAM during the initial DMA ramp ----
            warm_ps = psV.tile([P, 64], F32, tag="warm", bufs=1, name="warmps")
            for wi in range(220):
                nc.tensor.matmul(warm_ps[:], ones[:], ones[:, :64],
                                 start=(wi == 0), stop=(wi == 219))

            q_sb = {}
            k_sb = {}

            def alloc_qk(h):
                q_sb[h] = qkp.tile([P, T], F16, tag="q", name=f"qsb{h}")
                k_sb[h] = qkp.tile([P, T], F16, tag="k", name=f"ksb{h}")

            def rope_emit(ps, h, ci, which):
                # dst = ps*cos + shuffle(ps)*sin  (sin mask carries the signs)
                dst = (q_sb if which == "q" else k_sb)[h]
                a = rpp.tile([P, CH], F16, tag="ra")
                nc.vector.tensor_mul(a[:], ps[:], cos_sb[:, ts(ci, CH)])
                b = rpp.tile([P, CH], F32, tag="rb")
                nc.vector.stream_shuffle(b[:], ps[:], SWAP_MASK)
                b2 = rpp.tile([P, CH], F16, tag="rb2")
                nc.vector.tensor_mul(b2[:], b[:], sin_sb[:, ts(ci, CH)])
                nc.vector.tensor_add(dst[:, ts(ci, CH)], a[:], b2[:])

            def proj_mms(h, ci, wt, which, group):
                # group g in 0..3: emit contraction mms kt = 4g..4g+3
                if group == 0:
                    ps = psQK.tile([P, CH], F32, tag="qk",
                                   name=f"ps{which}{h}_{ci}")
                    proj_mms.cur = ps
                ps = proj_mms.cur
                for kt in range(4 * group, 4 * group + 4):
                    nc.tensor.matmul(ps[:], wt[:, kt, :],
                                     xs[kt][:, ts(ci, CH)],
                                     start=(kt == 0), stop=(kt == KT - 1))
                if group == 3:
                    rope_emit(ps, h, ci, which)

            def qk_proj_full(h, ci):
                for which, store in (("q", wq_h), ("k", wk_h)):
                    for g in range(4):
                        proj_mms(h, ci, store[h], which, g)

            def vproj_chunk(ci):
                for sub in range(4):
                    ti = 4 * ci + sub
                    for vc in range(2):
                        ps = psV.tile([P, CH], F32, tag="mmv")
                        for kt in range(KT):
                            nc.tensor.matmul(
                                ps[:],
                                xs[kt][:, ts(ti, P)],
                                wvt[kt][:, ts(vc, CH)],
                                start=(kt == 0), stop=(kt == KT - 1),
                            )
                        nc.scalar.copy(
                            v_sb[:, 4 * vc:4 * (vc + 1), ts(ti, P)],
                            ps[:].rearrange("p (j d) -> p j d", j=4),
                        )

            # ---- phase 1: vproj + qk proj of head 0 ----
            alloc_qk(0)
            for ci in range(NCH):
                qk_proj_full(0, ci)
                vproj_chunk(ci)
            es1.close()

            # attention-phase SBUF pools: created only after the wv pool is
            # freed (their charge windows don't overlap phase 1's peak), on
            # the right side so es_x can still close LIFO-style later.
            ynp = outer.enter_context(
                tc.tile_pool(name="ynorm", bufs=1, side="right"))
            ep = outer.enter_context(
                tc.tile_pool(name="ee", bufs=3, side="right"))
            esp = outer.enter_context(
                tc.tile_pool(name="esum", bufs=2, side="right"))
            rcp = outer.enter_context(
                tc.tile_pool(name="rc", bufs=2, side="right"))
            obp = outer.enter_context(
                tc.tile_pool(name="ost", bufs=3, side="right"))
            wpe = outer.enter_context(
                tc.tile_pool(name="wpE", bufs=1, side="right"))

            # attention-phase psum pools (stacked above psQK)
            es_attn = ExitStack()
            psS = es_attn.enter_context(tc.tile_pool(name="psS", bufs=2,
                                                     space="PSUM"))
            psY = es_attn.enter_context(tc.tile_pool(name="psY", bufs=2,
                                                     space="PSUM"))

            ynorm = [None] * HPC
            wpt = [None] * HPC
            wpp = None
            out_mm_queue = []  # deferred outproj (ti, oc) pairs

            def outproj_unit(ti, oc):
                ps = psQK.tile([P, CH], F32, tag="qk", name=f"pso{ti}_{oc}")
                for hh in range(HPC):
                    nc.tensor.matmul(
                        ps[:],
                        ynorm[hh][:, ts(ti, P)],
                        wpt[hh][:, ts(oc, CH)],
                        start=(hh == 0), stop=(hh == HPC - 1),
                    )
                ob = obp.tile([P, CH], F32, tag="ob")
                nc.vector.tensor_copy(ob[:], ps[:])
                # alternate output writes across the two HWDGE queues
                # (sync + scalar): one queue's ~264GB/s can't keep up with
                # the out-proj matmul rate, and the kernel ends at the
                # last DMA, not the last matmul
                eng = nc.sync if (ti * NCH + oc) % 2 == 0 else nc.scalar
                eng.dma_start(out[ts(ti, P), ts(oc, CH)], ob[:])

            pending = None  # (h, ci, y_ps, esum) awaiting denom/normalize

            def flush_pending():
                nonlocal pending
                if pending is None:
                    return
                ph, pci, py, pesum = pending
                pending = None
                d2 = psS.tile([P, 2 * CH], F32, tag="s", name="dps")
                d_ps = d2[:, ts(0, CH)]
                nc.tensor.matmul(d_ps, ones[:], pesum[:],
                                 start=True, stop=True)
                rc = rcp.tile([P, CH], F32, tag="rc")
                nc.vector.reciprocal_approx_fast(rc[:], d_ps)
                nc.vector.tensor_mul(ynorm[ph][:, ts(pci, CH)], py[:], rc[:])

            # ---- attention units: attn(h, ci) + qkproj(h+1, ci) ----
            for h in range(HPC):
                ynorm[h] = ynp.tile([P, T], F16, tag=f"yn{h}", name=f"ynorm{h}")
                if h + 1 < HPC:
                    alloc_qk(h + 1)
                if h + 2 < HPC:
                    load_wqk(h + 2)
                if h == HPC - 2:
                    # first two output-proj weight slices, prefetched early
                    for hh in range(2):
                        wt = wpe.tile([P, C], F16, tag=f"wpe{hh}",
                                      name=f"wpte{hh}")
                        nc.sync.dma_start(wt[:], wp[ts(hh, P), :])
                        wpt[hh] = wt
                last = h == HPC - 1
                if last:
                    # x no longer needed; reuse its SBUF for the wp weights
                    es_x.close()
                    wpp = tc.alloc_tile_pool(name="wp", bufs=1)
                    for hh in range(2, HPC):
                        wt = wpp.tile([P, C], F16, tag=f"wp{hh}",
                                      name=f"wpt{hh}")
                        nc.sync.dma_start(wt[:], wp[ts(hh, P), :])
                        wpt[hh] = wt
                    out_mm_queue.extend(
                        (ti, oc) for ti in range(TT) for oc in range(NCH))
                for ci in range(NCH):
                    y_ps = psY.tile([P, CH], F32, tag="y")
                    s_tiles = {}

                    def scores_pair(pt2):
                        # one [P, 2*CH] psum tile (2 banks); each matmul
                        # writes its own bank; exp reads the pair at once
                        s2 = psS.tile([P, 2 * CH], F32, tag="s",
                                      name=f"s{pt2}")
                        for j in range(2):
                            nc.tensor.matmul(
                                s2[:, ts(j, CH)], k_sb[h][:, ts(2 * pt2 + j, P)],
                                q_sb[h][:, ts(ci, CH)], start=True, stop=True,
                            )
                        s_tiles[pt2] = s2

                    scores_pair(0)
                    esum = esp.tile([P, CH], F16, tag="esum")
                    for pt in range(TT // 2):
                        e2 = ep.tile([P, 2 * CH], F16, tag="e", name="e2")
                        nc.scalar.activation(
                            e2[:], s_tiles.pop(pt)[:], Exp, scale=SCALE)
                        es = [e2[:, ts(0, CH)], e2[:, ts(1, CH)]]
                        if pt + 1 < TT // 2:
                            scores_pair(pt + 1)
                        for j in range(2):
                            kt = 2 * pt + j
                            nc.tensor.matmul(
                                y_ps[:], v_sb[:, h, ts(kt, P)], es[j],
                                start=(kt == 0), stop=(kt == TT - 1),
                            )
                        # denominator accumulation off the PE:
                        # DVE pairs (fp16 2x), GpSimd running sum
                        pair = rpp.tile([P, CH], F16, tag="pair")
                        nc.vector.tensor_add(pair[:], es[0], es[1])
                        if pt == 0:
                            nc.gpsimd.tensor_copy(esum[:], pair[:])
                        else:
                            nc.gpsimd.tensor_add(esum[:], esum[:], pair[:])
                        # previous unit's denom/normalize, off the critical path
                        if pt == 1:
                            flush_pending()
                        # interleave: qk-proj of next head (or tail outproj)
                        if not last:
                            which = "q" if pt < 4 else "k"
                            store = wq_h if pt < 4 else wk_h
                            proj_mms(h + 1, ci, store[h + 1], which, pt % 4)
                        elif ci > 0 and out_mm_queue and 2 <= pt <= 5:
                            outproj_unit(*out_mm_queue.pop(0))
                    pending = (h, ci, y_ps, esum)

            flush_pending()
            # ---- remaining out-projection ----
            for ti, oc in out_mm_queue:
                outproj_unit(ti, oc)
            es_attn.close()
            if wpp is not None:
                wpp.release()

    nc.compile()
    return nc


def get_nc():
    global _CACHED_NC
    if _CACHED_NC is None:
        _CACHED_NC = build_nc()
    return _CACHED_NC


def make_rope_masks():
    half = D // 2
    inv = 1.0 / (ROPE_BASE ** (np.arange(half, dtype=np.float64) * 2.0 / D))
    ang = np.arange(T, dtype=np.float64)[:, None] * inv[None, :]  # [T, half]
    cos = np.cos(ang).T.astype(np.float32)  # [half, T]
    sin = np.sin(ang).T.astype(np.float32)
    cosm = np.empty((P, T), np.float32)
    sinm = np.empty((P, T), np.float32)
    cosm[0::2] = cos
    cosm[1::2] = cos
    sinm[0::2] = -sin
    sinm[1::2] = sin
    return cosm.astype(np.float16), sinm.astype(np.float16)


def make_in_maps(x, w_attn, w_proj):
    x = np.asarray(x, dtype=np.float32)
    w_attn = np.asarray(w_attn, dtype=np.float32)
    w_proj = np.asarray(w_proj, dtype=np.float32)
    cosm, sinm = make_rope_masks()
    in_maps = []
    for core in range(8):
        b, hg = core // 2, core % 2
        h0 = hg * HPC
        rq = slice(h0 * D, (h0 + HPC) * D)
        rk = slice(C + h0 * D, C + (h0 + HPC) * D)
        rv = slice(2 * C + h0 * D, 2 * C + (h0 + HPC) * D)
        in_maps.append({
            "xt": np.ascontiguousarray(x[b].T).astype(np.float16),
            "wq": np.ascontiguousarray(w_attn[rq].T).astype(np.float16),
            "wk": np.ascontiguousarray(w_attn[rk].T).astype(np.float16),
            "wv": np.ascontiguousarray(w_attn[rv].T).astype(np.float16),
            "wp": np.ascontiguousarray(
                w_proj[:, h0 * D:(h0 + HPC) * D].T).astype(np.float16),
            "cosm": cosm,
            "sinm": sinm,
        })
    return in_maps


def combine_outputs(results):
    B = 4
    out = np.empty((B, T, C), np.float32)
    for b in range(B):
        out[b] = results[2 * b]["out"] + results[2 * b + 1]["out"]
    return out


def kernel(x, w_attn, w_proj):
    from concourse.bass_utils import run_bass_kernel_spmd

    nc = get_nc()
    in_maps = make_in_maps(x, w_attn, w_proj)
    res = run_bass_kernel_spmd(nc, in_maps, list(range(8)))
    return combine_outputs(res.results)
